# revision 19
# baseline (speedup 1.0000x reference)
"""Trainium2 Bass kernel for nn_DigitCapsule (dynamic routing, 2 routing steps).

Math (per reference), with s1 := sum_{n,k} W[c,n,d,k] x[b,n,k] (un-normalized):
  o1 = squash(s1/10) = s1 * sqrt(q)/(100+q),   q = sum_d s1^2
  t[b,c,n]   = sum_k x[b,n,k] V[b,c,n,k],      V = sum_d o1[b,c,d] W[c,n,d,k]
  delta      = (t - mean_c t)/10               (softmax linearized; |t|~1e-2)
  s2         = 0.1*s1 + sum_{n,k} delta*x*W    (the T1 matmul)
  out        = squash(s2)

Per-core strategy (batch 64 per core, pure data parallel):
  - s1: 72 accumulated matmuls (stationary x2 k-slices, moving wfk slices).
  - V: per c-pair p, stationary o1z [32,128] (block-diag o1, fp8), moving
    Wcd fp8 [32, (k,n)] 512-col chunks -> psum, drained *2^-11 to bf16
    on ACT/GPSIMD.
  - prod/fold/tm: DVE scalar_tensor_tensor (4x perf mode, all-bf16-SBUF).
  - tm transpose to n-partitions: hardware DMA xbar transpose.
  - T1: per k: z_k = tm o x (DVE 4x), then 18 accumulated matmuls with
    wfk c-group stationaries; corr extracted from the c-diagonal.
"""

import sys

import numpy as np
import ml_dtypes

if "/opt/trn_rl_repo" not in sys.path:
    sys.path.insert(0, "/opt/trn_rl_repo")

BF16NP = ml_dtypes.bfloat16
FP8NP = ml_dtypes.float8_e4m3

B = 512
NCORES = 8
BL = B // NCORES          # 64 batch per core
C = 10
N = 1152
D = 16
K = 8
NJ = N // 128             # 9 capsule chunks of 128
DC = D * C                # 160
KN = K * N                # 9216

WSCALE = 128.0            # fp8 W pre-scale
OSCALE = 16.0             # fp8 o1 pre-scale
DRAIN = 0.1 / (WSCALE * OSCALE)   # extra 0.1: t_all = t/10

_prog_cache = {}


def build_program(stage=0):
    if stage in _prog_cache:
        return _prog_cache[stage]

    from contextlib import ExitStack
    import concourse.bacc as bacc
    import concourse.tile as tile
    import concourse.mybir as mybir

    F32 = mybir.dt.float32
    BF16 = mybir.dt.bfloat16
    FP8 = mybir.dt.float8e4
    ADD = mybir.AluOpType.add
    SUB = mybir.AluOpType.subtract
    MULT = mybir.AluOpType.mult
    AF = mybir.ActivationFunctionType

    nc = bacc.Bacc()

    xz2_d = nc.dram_tensor("xz2", [128, K, NJ * BL], BF16, kind="ExternalInput")
    xbb_d = nc.dram_tensor("xbb", [128, K, N], BF16, kind="ExternalInput")
    wfk_d = nc.dram_tensor("wfk", [128, NJ, K, DC], BF16, kind="ExternalInput")
    wcd_d = nc.dram_tensor("wcd", [128, 2, KN], FP8, kind="ExternalInput")
    dltc_d = nc.dram_tensor("dltc", [128, BL], BF16, kind="ExternalInput")
    rep2_d = nc.dram_tensor("rep2", [BL, 128], BF16, kind="ExternalInput")
    id128_d = nc.dram_tensor("id128", [128, 128], BF16, kind="ExternalInput")
    msk_d = nc.dram_tensor("msk", [BL, 2, 32], F32, kind="ExternalInput")
    idf_d = nc.dram_tensor("idf", [128, 128], F32, kind="ExternalInput")
    out_d = nc.dram_tensor("out", [BL, DC], F32, kind="ExternalOutput")

    with tile.TileContext(nc) as tc, ExitStack() as ctx:
        const = ctx.enter_context(tc.tile_pool(name="const", bufs=1))
        small = ctx.enter_context(tc.tile_pool(name="small", bufs=1))
        vsb_pool = ctx.enter_context(tc.tile_pool(name="vsb", bufs=2))
        zpool = ctx.enter_context(tc.tile_pool(name="zp", bufs=2))
        ps_acc = ctx.enter_context(tc.tile_pool(name="ps_acc", bufs=1, space="PSUM"))
        ps_v = ctx.enter_context(tc.tile_pool(name="ps_v", bufs=2, space="PSUM"))
        ps_t1 = ctx.enter_context(tc.tile_pool(name="ps_t1", bufs=1, space="PSUM"))
        ps_mi = ctx.enter_context(tc.tile_pool(name="ps_mi", bufs=1, space="PSUM"))

        # ---- input tiles + loads ----
        xz2 = const.tile([128, K, NJ * BL], BF16)
        xbb = const.tile([128, K, N], BF16)
        wfk = const.tile([128, NJ, K, DC], BF16)
        wcd = const.tile([128, 2, KN], FP8)
        dltc = const.tile([128, BL], BF16)
        rep2 = const.tile([BL, 128], BF16)
        id128 = const.tile([128, 128], BF16)
        msk = const.tile([BL, 2, 32], F32)
        idf = const.tile([128, 128], F32)

        for j in range(NJ):
            nc.sync.dma_start(wfk[:, j], wfk_d[:, j])
        for k in range(K):
            nc.sync.dma_start(xz2[:, k], xz2_d[:, k])
        for g in range(4):
            s = slice(2304 * g, 2304 * (g + 1))
            nc.sync.dma_start(wcd[:, 0, s], wcd_d[:, 0, s])
            nc.sync.dma_start(xbb[:, 2 * g:2 * g + 2, :], xbb_d[:, 2 * g:2 * g + 2, :])
        nc.sync.dma_start(wcd[0:32, 1, :], wcd_d[0:32, 1, :])
        nc.sync.dma_start(dltc[:], dltc_d[:])
        nc.sync.dma_start(rep2[:], rep2_d[:])
        nc.sync.dma_start(id128[:], id128_d[:])
        nc.sync.dma_start(msk[:], msk_d[:])
        nc.sync.dma_start(idf[:], idf_d[:])

        # ---- s1 [64, (c,d)] : 72 accumulated matmuls ----
        s1_ps = ps_acc.tile([BL, DC], F32, name="s1_ps", tag="acc")
        for j in range(NJ):
            for k in range(K):
                nc.tensor.matmul(s1_ps[:], xz2[:, k, BL * j:BL * (j + 1)], wfk[:, j, k, :],
                                 start=(j == 0 and k == 0),
                                 stop=(j == NJ - 1 and k == K - 1))
        s1sb = small.tile([BL, DC], F32)
        nc.scalar.copy(s1sb[:], s1_ps[:])

        if stage == 1:
            nc.sync.dma_start(out_d[:], s1sb[:])
            nc.compile()
            _prog_cache[stage] = nc
            return nc

        # ---- o1 = squash(s1/10) = s1*sqrt(q)/(100+q) ----
        s1v = s1sb[:].rearrange("b (c d) -> b c d", c=C, d=D)
        sq = small.tile([BL, C, D], F32)
        nc.vector.tensor_tensor(sq[:], s1v, s1v, MULT)
        q8 = small.tile([BL, C, 8], F32)
        nc.vector.tensor_tensor(q8[:], sq[:, :, 0:8], sq[:, :, 8:16], ADD)
        q4 = small.tile([BL, C, 4], F32)
        nc.vector.tensor_tensor(q4[:], q8[:, :, 0:4], q8[:, :, 4:8], ADD)
        q2 = small.tile([BL, C, 2], F32)
        nc.vector.tensor_tensor(q2[:], q4[:, :, 0:2], q4[:, :, 2:4], ADD)
        q1 = small.tile([BL, C], F32)
        nc.vector.tensor_tensor(q1[:], q2[:, :, 0], q2[:, :, 1], ADD)
        sqrtq = small.tile([BL, C], F32)
        nc.scalar.activation(sqrtq[:], q1[:], AF.Sqrt)
        den = small.tile([BL, C], F32)
        nc.vector.tensor_scalar_add(den[:], q1[:], 100.0)
        rden = small.tile([BL, C], F32)
        nc.vector.reciprocal(rden[:], den[:])
        g1 = small.tile([BL, C], F32)
        nc.vector.tensor_tensor(g1[:], sqrtq[:], rden[:], MULT)
        o1sb = small.tile([BL, DC], F32)
        nc.vector.tensor_tensor(
            o1sb[:].rearrange("b (c d) -> b c d", c=C, d=D), s1v,
            g1[:].unsqueeze(2).broadcast_to((BL, C, D)), MULT)

        # ---- o1z: per pair p, [32, 128] block-diag fp8 at partition 32*(p%4) ----
        # o1masked[b, p, c2, (c2',d)] = OSCALE * o1[b, 2p+c2', d] * [c2==c2']
        o1masked = small.tile([BL, 5, 2, 32], F32)
        for c2 in range(2):
            nc.vector.scalar_tensor_tensor(
                o1masked[:, :, c2, :],
                o1sb[:].rearrange("b (p q) -> b p q", p=5, q=32),
                OSCALE,
                msk[:, c2, :].unsqueeze(1).broadcast_to((BL, 5, 32)),
                MULT, MULT)
        o1z = const.tile([128, 2, 128], FP8)
        o1z_ps = ps_mi.tile([128, 2, 128], F32, name="o1z_ps", tag="mi")
        for p in range(5):
            r, blk = (p % 4) * 32, p // 4
            for c2 in range(2):
                nc.tensor.matmul(o1z_ps[r:r + 32, blk, BL * c2:BL * (c2 + 1)],
                                 o1masked[:, p, c2, :], idf[0:BL, 0:BL],
                                 start=True, stop=True, tile_position=(0, r))
            nc.scalar.copy(o1z[r:r + 32, blk, :], o1z_ps[r:r + 32, blk, :])

        # ---- V per pair -> Vsb bf16; prod/fold on DVE -> t_all ----
        t_all = const.tile([128, 5, N], BF16, name="t_all", tag="tt")
        for p in range(5):
            r, blk = (p % 4) * 32, p // 4
            vsb = vsb_pool.tile([128, KN], BF16, name="vsb")
            for qd in range(9):  # 9 double-chunks of 1024
                vq = ps_v.tile([128, 2, 512], F32, name="vq")
                for h in range(2):
                    qq = 2 * qd + h
                    nc.tensor.matmul(vq[:, h, :], o1z[r:r + 32, blk, :],
                                     wcd[r:r + 32, blk, 512 * qq:512 * (qq + 1)],
                                     start=True, stop=True, tile_position=(r, 0))
                dst = vsb[:, 1024 * qd:1024 * (qd + 1)]
                src = vq[:].rearrange("p t f -> p (t f)")
                nc.scalar.mul(dst, src, DRAIN)
            prodsb = vsb_pool.tile([128, KN], BF16, name="prodsb", tag="pw", bufs=1)
            nc.vector.tensor_tensor(
                prodsb[:], vsb[:],
                xbb[:].rearrange("p k n -> p (k n)"), MULT)
            fa = zpool.tile([128, KN // 2], BF16, name="fa", tag="fa", bufs=1)
            nc.vector.tensor_tensor(
                fa[:], prodsb[:, 0:KN // 2], prodsb[:, KN // 2:KN], ADD)
            fb = vsb_pool.tile([128, KN // 4], BF16, name="fb", tag="pw", bufs=1)
            nc.vector.tensor_tensor(
                fb[:], fa[:, 0:KN // 4], fa[:, KN // 4:KN // 2], ADD)
            nc.vector.tensor_tensor(
                t_all[:, p, :], fb[:, 0:N], fb[:, N:2 * N], ADD)

        if stage == 2:
            to = small.tile([BL, DC], F32)
            nc.vector.tensor_copy(to[:], t_all[0:BL, 0, 0:DC])
            nc.sync.dma_start(out_d[:], to[:])
            nc.compile()
            _prog_cache[stage] = nc
            return nc

        # ---- tbar = 0.01 * sum_c t  (PE fold over (c2, pairs)), replicated ----
        tbarsb = small.tile([BL, N], BF16)
        tbrsb = small.tile([128, N], BF16)
        ccs = [(0, 512), (512, 512), (1024, 128)]
        for (o, w) in ccs:
            tb_ps = ps_acc.tile([BL, 512], F32, name="tb_ps", tag="acc")
            for p in range(5):
                nc.tensor.matmul(tb_ps[:, 0:w], dltc[:], t_all[:, p, o:o + w],
                                 start=(p == 0), stop=(p == 4))
            nc.scalar.copy(tbarsb[:, o:o + w], tb_ps[:, 0:w])
        for (o, w) in ccs:
            tr_ps = ps_acc.tile([128, 512], F32, name="tr_ps", tag="acc")
            nc.tensor.matmul(tr_ps[:, 0:w], rep2[:], tbarsb[:, o:o + w],
                             start=True, stop=True)
            nc.scalar.copy(tbrsb[:, o:o + w], tr_ps[:, 0:w])

        # ---- tm = 0.1*t - tbarR  (= softmax delta, already /10) ----
        tm_all = const.tile([128, 5, N], BF16)
        tmT2 = const.tile([128, C, NJ * BL], BF16, name="tmT2", tag="tt")
        for p in range(5):
            nc.vector.tensor_tensor(
                tm_all[:, p, :], t_all[:, p, :], tbrsb[:], SUB)
            for c2 in range(2):
                nc.sync.dma_start_transpose(
                    tmT2[:, 2 * p + c2, :].rearrange("m (j b) -> m j b", j=NJ),
                    tm_all[BL * c2:BL * (c2 + 1), p, :])

        # ---- z_k = tm o x ; T1 accumulated matmuls, c-diag extraction ----
        t1a_ps = ps_t1.tile([80, 320], F32, name="t1a_ps")
        t1b_ps = ps_t1.tile([80, 320], F32, name="t1b_ps")
        for k in range(K):
            zk = zpool.tile([128, C, NJ * BL], BF16, name="zk")
            nc.vector.tensor_tensor(
                zk[:], tmT2[:],
                xz2[:, k, :].unsqueeze(1).broadcast_to((128, C, NJ * BL)),
                MULT)
            for j in range(NJ):
                st = (k == 0 and j == 0)
                sp = (k == K - 1 and j == NJ - 1)
                nc.tensor.matmul(t1a_ps[:], wfk[:, j, k, 0:80],
                                 zk[:, 0:5, BL * j:BL * (j + 1)], start=st, stop=sp)
                nc.tensor.matmul(t1b_ps[:], wfk[:, j, k, 80:160],
                                 zk[:, 5:10, BL * j:BL * (j + 1)], start=st, stop=sp)

        t1a_sb = small.tile([80, 320], F32)
        t1b_sb = small.tile([80, 320], F32)
        nc.scalar.copy(t1a_sb[:], t1a_ps[:])
        nc.scalar.copy(t1b_sb[:], t1b_ps[:])

        # ---- s2 = 0.1*s1 + corr (c-diagonal of T1) ; out = squash(s2) ----
        # transpose T1 column-chunks; for chunk g of group t, the diagonal of
        # c = 5*t + 2*g + u sits at out partitions [64u:64u+64], cols [16u:...]
        s2sb = small.tile([BL, DC], F32)
        for t, src in ((0, t1a_sb), (1, t1b_sb)):
            for g in range(3):
                w = 128 if g < 2 else 64
                ct_ps = ps_mi.tile([128, 80], F32, name="ct_ps", tag="mi")
                nc.tensor.transpose(ct_ps[0:w, :], src[:, 128 * g:128 * g + w],
                                    idf[0:80, 0:80])
                for u in range(w // BL):
                    cl = 2 * g + u           # index within the 5-c group
                    c = 5 * t + cl           # global c
                    nc.vector.scalar_tensor_tensor(
                        s2sb[:, 16 * c:16 * (c + 1)],
                        s1sb[:, 16 * c:16 * (c + 1)], 0.1,
                        ct_ps[BL * u:BL * (u + 1), 16 * cl:16 * (cl + 1)],
                        MULT, ADD)
        s2v = s2sb[:].rearrange("b (c d) -> b c d", c=C, d=D)
        sq2 = small.tile([BL, C, D], F32)
        nc.vector.tensor_tensor(sq2[:], s2v, s2v, MULT)
        p8 = small.tile([BL, C, 8], F32)
        nc.vector.tensor_tensor(p8[:], sq2[:, :, 0:8], sq2[:, :, 8:16], ADD)
        p4 = small.tile([BL, C, 4], F32)
        nc.vector.tensor_tensor(p4[:], p8[:, :, 0:4], p8[:, :, 4:8], ADD)
        p2 = small.tile([BL, C, 2], F32)
        nc.vector.tensor_tensor(p2[:], p4[:, :, 0:2], p4[:, :, 2:4], ADD)
        p1 = small.tile([BL, C], F32)
        nc.vector.tensor_tensor(p1[:], p2[:, :, 0], p2[:, :, 1], ADD)
        sq2r = small.tile([BL, C], F32)
        nc.scalar.activation(sq2r[:], p1[:], AF.Sqrt)
        den2 = small.tile([BL, C], F32)
        nc.vector.tensor_scalar_add(den2[:], p1[:], 1.0)
        rden2 = small.tile([BL, C], F32)
        nc.vector.reciprocal(rden2[:], den2[:])
        g2 = small.tile([BL, C], F32)
        nc.vector.tensor_tensor(g2[:], sq2r[:], rden2[:], MULT)
        outv = small.tile([BL, DC], F32)
        nc.vector.tensor_tensor(
            outv[:].rearrange("b (c d) -> b c d", c=C, d=D), s2v,
            g2[:].unsqueeze(2).broadcast_to((BL, C, D)), MULT)
        nc.sync.dma_start(out_d[:], outv[:])

    nc.compile()
    _prog_cache[stage] = nc
    return nc


def _prep_shared(weight):
    # wfk[m, j, k, c*16+d] = W[c, 128j+m, d, k]
    w = weight.astype(np.float32)                       # [C, N, D, K]
    wfk = np.ascontiguousarray(
        w.transpose(1, 3, 0, 2)                         # [N, K, C, D]
        .reshape(NJ, 128, K, C * D)
        .transpose(1, 0, 2, 3)                          # [128, NJ, K, CD]
    ).astype(BF16NP)
    # wcd[(32*(p%4) + 16*c2 + d), p//4, k*N+n] = W[2p+c2, n, d, k] * WSCALE
    wcd_full = (w.transpose(0, 2, 3, 1)                 # [C, D, K, N]
                .reshape(C, D, KN) * WSCALE)            # [c, d, (k,n)] k-major
    wcd = np.zeros((128, 2, KN), dtype=np.float32)
    for p in range(5):
        r, blk = (p % 4) * 32, p // 4
        for c2 in range(2):
            wcd[r + 16 * c2: r + 16 * (c2 + 1), blk] = wcd_full[2 * p + c2]
    wcd = wcd.astype(FP8NP)
    dltc = np.tile(0.1 * np.eye(BL, dtype=np.float32), (2, 1)).astype(BF16NP)
    rep2 = np.tile(np.eye(BL, dtype=np.float32), (1, 2)).astype(BF16NP)
    id128 = np.eye(128, dtype=np.float32).astype(BF16NP)
    idf = np.eye(128, dtype=np.float32)
    # msk[b, c2, 16*c2' + d] = [c2' == c2]
    msk = np.broadcast_to(
        np.kron(np.eye(2, dtype=np.float32),
                np.ones((16,), dtype=np.float32)).reshape(1, 2, 32),
        (BL, 2, 32)).astype(np.float32).copy()
    return wfk, wcd, dltc, rep2, id128, msk, idf


def _prep_x_shard(xs):
    xf = xs.astype(np.float32)                          # [BL, N, K]
    # xz2[m, k, 64*j + b] = x[b, 128j+m, k]
    xz2 = np.ascontiguousarray(
        xf.transpose(1, 2, 0).reshape(NJ, 128, K, BL).transpose(1, 2, 0, 3)
        .reshape(128, K, NJ * BL)
    ).astype(BF16NP)
    xkn = xf.transpose(0, 2, 1)                         # [BL, K, N]
    xbb = np.ascontiguousarray(
        np.concatenate([xkn, xkn], axis=0)              # [(c2,b)=128, K, N]
    ).astype(BF16NP)
    return xz2, xbb


def _make_inmaps(x, weight):
    wfk, wcd, dltc, rep2, id128, msk, idf = _prep_shared(weight)
    in_maps = []
    for core in range(NCORES):
        xs = x[core * BL:(core + 1) * BL]
        xz2, xbb = _prep_x_shard(xs)
        in_maps.append({"xz2": xz2, "xbb": xbb, "wfk": wfk, "wcd": wcd,
                        "dltc": dltc, "rep2": rep2, "id128": id128,
                        "msk": msk, "idf": idf})
    return in_maps


def _postprocess(o):
    # o: [BL, (c,d)] c-major -> [BL, C, D]
    return np.asarray(o, dtype=np.float32).reshape(BL, C, D)


def kernel(x, weight):
    """x: [512, 1152, 8] f32; weight: [10, 1152, 16, 8] f32 -> [512, 10, 16] f32."""
    from concourse.bass_utils import run_bass_kernel_spmd

    nc = build_program()
    x = np.asarray(x, dtype=np.float32)
    weight = np.asarray(weight, dtype=np.float32)
    in_maps = _make_inmaps(x, weight)
    res = run_bass_kernel_spmd(nc, in_maps, list(range(NCORES)))
    outs = [_postprocess(res.results[core]["out"]) for core in range(NCORES)]
    return np.ascontiguousarray(np.concatenate(outs, axis=0))
